# revision 24
# baseline (speedup 1.0000x reference)
"""Trainium2 Bass kernel for ClippingAttentionEngine.

Full (unsharded) inputs in, full output out. Internally shards across 8
NeuronCores: batch (4-way) x head-group (2-way).  Each core computes
attention for one batch and 8 of the 16 heads, plus the row-parallel
partial of the output projection; the host sums the two head-group
partials per batch and adds bo.

Math notes (validated against the reference on the fixed inputs):
 - softmax_k(A + lam*prior) is shift-invariant per query, so the
   threshold subtraction cancels; the clip mask only removes entries
   whose softmax weight is < e^-20 relative to the row max, which is
   below fp32 resolution of the result.  The kernel therefore computes
   plain softmax(QK^T/sqrt(hd) + lam*prior).
 - exp is split as exp(A)*exp(lam*prior): exp(lam*prior) is shared by
   all 8 heads on a core and scaled by the runtime per-batch lam via
   the ACT per-partition scale operand.
 - scores are computed transposed (S^T[k,q]) so P^T feeds the O=P@V
   matmul directly; the softmax denominator rides as an extra ones
   column appended to V (O^T row 64).

Schedule notes (the kernel is ACT(exp)-throughput bound; everything
else is arranged to hide under the exp stream):
 - Phase B runs as one global slot stream over (block, kt) where a
   block is a (qc, hp) pair; the P@V accumulation for slot t-LAG is
   emitted right before the scores matmuls of slot t, so PE never
   waits on the scores->exp->mult chain of the current slot.
 - Blocks are ordered (q0,h0)(q1,h0)(q0,h1)(q1,h1)... so only two
   exp(lam*prior) chunks are resident and the Q/K projections for
   heads 1..3 plus the V projection stream as per-slot fillers inside
   the early blocks instead of serializing ahead of the attention.
 - The lambda chain and the head-0 Q/K projection are paced directly
   behind the x/dx DMA so exp work starts ~25us into the kernel.
 - Denominator reciprocals are broadcast across partitions on the
   otherwise idle GPSIMD engine; the output projection is emitted at
   each qc's drain so it overlaps the next blocks.
"""

import sys

sys.path.insert(0, "/opt/trn_rl_repo")

from contextlib import ExitStack

import numpy as np
import ml_dtypes

import concourse.bacc as bacc
import concourse.tile as tile
from concourse import mybir
from concourse import bass_utils

F32 = mybir.dt.float32
BF16 = mybir.dt.bfloat16
AF = mybir.ActivationFunctionType
OP = mybir.AluOpType
AX = mybir.AxisListType

B, S, D = 4, 2048, 1024
H, HD = 16, 64
N_CORES = 8
HPC = 8          # heads per core
GD = HPC * HD    # head-group width (512)
QC = 512         # q-chunk width
NQC = S // QC    # 4
NKT = S // 128   # 16 k-tiles
NDT = D // 128   # 8 d-tiles
NST = S // 128   # 16 s-tiles
NMT = GD // 128  # 4 m-tiles (head pairs)
VW = HD + 1      # V block width incl. denominator ones column
LAG = 4          # slots the P@V stream trails the scores stream by
LAMBDA_MAX = 10.0
ALPHA = 5.0
EPS = 1e-8

_CACHE = {}


def build_nc(loop_reps=None):
    nc = bacc.Bacc("TRN2", target_bir_lowering=False, debug=False,
                   num_devices=N_CORES)

    xT = nc.dram_tensor("xT", [D, S], BF16, kind="ExternalInput")
    dxT = nc.dram_tensor("dxT", [D, S], BF16, kind="ExternalInput")
    wqT = nc.dram_tensor("wqT", [D, GD], BF16, kind="ExternalInput")
    wkT = nc.dram_tensor("wkT", [D, GD], BF16, kind="ExternalInput")
    wvT = nc.dram_tensor("wvT", [D, GD], BF16, kind="ExternalInput")
    woT = nc.dram_tensor("woT", [GD, D], BF16, kind="ExternalInput")
    bq = nc.dram_tensor("bq", [1, GD], BF16, kind="ExternalInput")
    bk = nc.dram_tensor("bk", [1, GD], BF16, kind="ExternalInput")
    bv = nc.dram_tensor("bv", [1, GD], BF16, kind="ExternalInput")
    priorT = nc.dram_tensor("priorT", [S, S], F32, kind="ExternalInput")
    ident = nc.dram_tensor("ident", [128, 128], F32, kind="ExternalInput")
    out_p = nc.dram_tensor("out_p", [S, D], F32, kind="ExternalOutput")

    with tile.TileContext(nc) as tc, ExitStack() as st_outer:
        consts = st_outer.enter_context(tc.tile_pool(name="consts", bufs=1))
        qkv = st_outer.enter_context(tc.tile_pool(name="qkv", bufs=1))

        ones_row = consts.tile([1, QC], BF16, tag="ones_row")
        nc.vector.memset(ones_row, 1.0)
        ones_c32 = consts.tile([128, 1], F32, tag="ones_c32")
        nc.vector.memset(ones_c32, 1.0)
        ones_r32 = consts.tile([1, 128], F32, tag="ones_r32")
        nc.vector.memset(ones_r32, 1.0)
        ident_sb = consts.tile([128, 128], F32, tag="ident")
        nc.sync.dma_start(out=ident_sb, in_=ident.ap())
        bq_sb = consts.tile([1, GD], BF16, tag="bq")
        nc.sync.dma_start(out=bq_sb, in_=bq.ap())
        bk_sb = consts.tile([1, GD], BF16, tag="bk")
        nc.sync.dma_start(out=bk_sb, in_=bk.ap())
        bv_sb = consts.tile([1, GD], BF16, tag="bv")
        nc.sync.dma_start(out=bv_sb, in_=bv.ap())

        wq_sb = [consts.tile([128, GD], BF16, tag=f"wq{d}", name=f"wq{d}") for d in range(NDT)]
        wk_sb = [consts.tile([128, GD], BF16, tag=f"wk{d}", name=f"wk{d}") for d in range(NDT)]
        wv_sb = [consts.tile([128, GD], BF16, tag=f"wv{d}", name=f"wv{d}") for d in range(NDT)]
        for d in range(NDT):
            nc.sync.dma_start(out=wq_sb[d], in_=wqT.ap()[d * 128:(d + 1) * 128, :])
            nc.sync.dma_start(out=wk_sb[d], in_=wkT.ap()[d * 128:(d + 1) * 128, :])
            nc.sync.dma_start(out=wv_sb[d], in_=wvT.ap()[d * 128:(d + 1) * 128, :])
        wo_sb = [consts.tile([128, D], BF16, tag=f"wo{c}", name=f"wo{c}") for c in range(NMT)]
        for c in range(NMT):
            nc.sync.dma_start(out=wo_sb[c], in_=woT.ap()[c * 128:(c + 1) * 128, :])

        QT = [qkv.tile([128, S], BF16, tag=f"QT{m}", name=f"QT{m}") for m in range(NMT)]
        KT = [qkv.tile([128, S], BF16, tag=f"KT{m}", name=f"KT{m}") for m in range(NMT)]
        VH = qkv.tile([128, NKT * VW * HPC], BF16, tag="VH")
        nc.vector.memset(VH, 1.0)
        OT = [[qkv.tile([128, QC], BF16, tag=f"OT{c}_{q}", name=f"OT{c}_{q}")
               for q in range(NQC)] for c in range(NMT)]
        lam_bc = consts.tile([128, 1], F32, tag="lam_bc")

        def body():
            # ---------------- Phase A: x DMA, head-0 Q/K, lambda ----------
            with tc.tile_pool(name="xp", bufs=1) as xpool:
                x_sb = [xpool.tile([128, S], BF16, tag=f"x{d}", name=f"x{d}")
                        for d in range(NDT)]
                for d in range(NDT):
                    nc.sync.dma_start(out=x_sb[d],
                                      in_=xT.ap()[d * 128:(d + 1) * 128, :])

                # mt=0 Q/K paced behind the x DMA (d-major so the first
                # matmuls only need x0)
                with tc.tile_pool(name="projA", bufs=8, space="PSUM") as projA:
                    psA = [projA.tile([128, QC], F32, tag="projA",
                                      name=f"psA{g}") for g in range(8)]
                    for d in range(NDT):
                        for g in range(8):
                            dst_i, sc = g // 4, g % 4
                            w_sb = wq_sb if dst_i == 0 else wk_sb
                            nc.tensor.matmul(
                                psA[g], w_sb[d][:, 0:128],
                                x_sb[d][:, sc * QC:(sc + 1) * QC],
                                start=(d == 0), stop=False)
                    for g in range(8):
                        dst_i, sc = g // 4, g % 4
                        dst = QT if dst_i == 0 else KT
                        b_sb = bq_sb if dst_i == 0 else bk_sb
                        nc.tensor.matmul(psA[g], b_sb[:, 0:128], ones_row,
                                         start=False, stop=True)
                        nc.vector.tensor_copy(dst[0][:, sc * QC:(sc + 1) * QC],
                                              psA[g])

                # lambda chain: row norms of x and dx via Gram diagonals
                with tc.tile_pool(name="dxp", bufs=1) as dxpool, \
                     tc.tile_pool(name="gr2", bufs=2, space="PSUM") as gr2, \
                     tc.tile_pool(name="tyA", bufs=1, space="PSUM") as tyA, \
                     tc.tile_pool(name="miscA", bufs=2) as miscA:

                    dx_sb = [dxpool.tile([128, S], BF16, tag=f"dxt{d}",
                                         name=f"dxt{d}") for d in range(NDT)]
                    for d in range(NDT):
                        nc.sync.dma_start(
                            out=dx_sb[d],
                            in_=dxT.ap()[d * 128:(d + 1) * 128, :])

                    nx2 = miscA.tile([128, NST], F32, tag="nx2")
                    ndx2 = miscA.tile([128, NST], F32, tag="ndx2")
                    for arr_sb, acc in ((x_sb, nx2), (dx_sb, ndx2)):
                        for s_t in range(NST):
                            psg = gr2.tile([128, 128], F32, tag="gram")
                            for d in range(NDT):
                                sl = arr_sb[d][:, s_t * 128:(s_t + 1) * 128]
                                nc.tensor.matmul(psg, sl, sl, start=(d == 0),
                                                 stop=(d == NDT - 1))
                            diag = miscA.tile([128, 128], F32, tag="diag")
                            nc.vector.tensor_tensor(diag, psg, ident_sb,
                                                    OP.mult)
                            nc.vector.tensor_reduce(acc[:, s_t:s_t + 1], diag,
                                                    axis=AX.X, op=OP.add)

                    nx = miscA.tile([128, NST], F32, tag="nx")
                    ndx = miscA.tile([128, NST], F32, tag="ndx")
                    nc.scalar.activation(nx, nx2, AF.Sqrt)
                    nc.scalar.activation(ndx, ndx2, AF.Sqrt)
                    eps_sb = miscA.tile([128, 1], F32, tag="eps")
                    nc.vector.memset(eps_sb, EPS)
                    nxe = miscA.tile([128, NST], F32, tag="nxe")
                    nc.scalar.activation(nxe, nx, AF.Identity, bias=eps_sb)
                    rx = miscA.tile([128, NST], F32, tag="rx")
                    nc.vector.reciprocal(rx, nxe)
                    u = miscA.tile([128, NST], F32, tag="u")
                    nc.vector.tensor_tensor(u, ndx, rx, OP.mult)
                    usum = miscA.tile([128, 1], F32, tag="usum")
                    nc.vector.tensor_reduce(usum, u, axis=AX.X, op=OP.add)
                    ps_u = tyA.tile([1, 1], F32, tag="psu")
                    nc.tensor.matmul(ps_u, usum, ones_c32, start=True,
                                     stop=True)
                    lam1 = miscA.tile([1, 1], F32, tag="lam1")
                    nc.scalar.activation(lam1, ps_u, AF.Exp, scale=-ALPHA / S)
                    ps_l = tyA.tile([128, 1], F32, tag="psl")
                    nc.tensor.matmul(ps_l, ones_r32, lam1, start=True,
                                     stop=True)
                    nc.scalar.mul(lam_bc, ps_l, LAMBDA_MAX)

                # -------------- Phase B: global slot stream --------------
                # blocks: (qc, hp) in order (q0,h0)(q1,h0)(q0,h1)(q1,h1)...
                # then the q2/q3 section.
                blocks = []
                for qp in range(2):
                    for hp in range(NMT):
                        for j in range(2):
                            blocks.append((2 * qp + j, hp))
                nslots = len(blocks) * NKT

                # fillers[slot] = list of thunks emitting PE work
                fillers = [[] for _ in range(nslots)]

                def emit_v_group(pool, s_t):
                    ps = pool.tile([128, QC], F32, tag="proj", name="proj")
                    for d in range(NDT):
                        nc.tensor.matmul(
                            ps, x_sb[d][:, s_t * 128:(s_t + 1) * 128],
                            wv_sb[d], start=(d == 0), stop=False)
                    nc.tensor.matmul(ps, ones_row[:, 0:128], bv_sb,
                                     start=False, stop=True)
                    base = s_t * VW * HPC
                    dst3 = VH[:, base:base + VW * HPC].rearrange(
                        "p (h c) -> p h c", c=VW)[:, :, 0:HD]
                    src3 = ps.rearrange("p (h c) -> p h c", c=HD)
                    nc.vector.tensor_copy(dst3, src3)

                with tc.tile_pool(name="ps2", bufs=2, space="PSUM") as ps2, \
                     tc.tile_pool(name="psov", bufs=3, space="PSUM") as psov, \
                     tc.tile_pool(name="projB", bufs=1, space="PSUM") as projB, \
                     tc.tile_pool(name="ebp", bufs=2) as ebp, \
                     tc.tile_pool(name="prp", bufs=3) as prp, \
                     tc.tile_pool(name="pap", bufs=2) as pap, \
                     tc.tile_pool(name="php", bufs=3) as php, \
                     tc.tile_pool(name="msp", bufs=2) as msp, \
                     tc.tile_pool(name="otp", bufs=1) as otp:

                    # V projection: one s-tile per slot in block 0
                    for kt in range(NKT):
                        fillers[kt].append(
                            lambda s_t=kt: emit_v_group(projB, s_t))
                    # Q/K mt=1..3: 8 groups each, spread 2/4/4 slots apart
                    for mt, (b0, step) in ((1, (16, 2)), (2, (32, 4)),
                                           (3, (64, 4))):
                        for g in range(8):
                            dst_i, sc = g // 4, g % 4

                            def f(mt=mt, dst_i=dst_i, sc=sc):
                                dst = QT if dst_i == 0 else KT
                                w_sb = wq_sb if dst_i == 0 else wk_sb
                                b_sb = bq_sb if dst_i == 0 else bk_sb
                                ps = projB.tile([128, QC], F32, tag="proj",
                                                name="proj")
                                for d in range(NDT):
                                    nc.tensor.matmul(
                                        ps,
                                        w_sb[d][:, mt * 128:(mt + 1) * 128],
                                        x_sb[d][:, sc * QC:(sc + 1) * QC],
                                        start=(d == 0), stop=False)
                                nc.tensor.matmul(
                                    ps, b_sb[:, mt * 128:(mt + 1) * 128],
                                    ones_row, start=False, stop=True)
                                nc.vector.tensor_copy(
                                    dst[mt][:, sc * QC:(sc + 1) * QC], ps)
                            fillers[b0 + step * g].append(f)

                    expB = {}          # qc -> expB tile
                    ph_ring = {}       # slot -> ph2 tile
                    pso_blk = {}       # block index -> [pso0, pso1]
                    done_hp = {qc: 0 for qc in range(NQC)}

                    def emit_expB(qc):
                        eb = ebp.tile([128, NKT * QC], BF16, tag="expB",
                                      name="expB")
                        for kt in range(NKT):
                            pr = prp.tile([128, QC], F32, tag="prior",
                                          name="prior")
                            nc.sync.dma_start(
                                out=pr,
                                in_=priorT.ap()[kt * 128:(kt + 1) * 128,
                                                qc * QC:(qc + 1) * QC])
                            nc.scalar.activation(
                                eb[:, kt * QC:(kt + 1) * QC], pr, AF.Exp,
                                scale=lam_bc)
                        expB[qc] = eb

                    pair_pa = {}

                    def emit_sc(t):
                        bi, kt = t // NKT, t % NKT
                        qc, hp = blocks[bi]
                        if kt == 0 and qc not in expB:
                            emit_expB(qc)
                        pss2 = ps2.tile([128, 2 * QC], F32, tag="pss2",
                                        name="pss2")
                        for i in range(2):
                            r0 = i * HD
                            nc.tensor.matmul(
                                pss2[:, i * QC:(i + 1) * QC],
                                KT[hp][r0:r0 + HD, kt * 128:(kt + 1) * 128],
                                QT[hp][r0:r0 + HD, qc * QC:(qc + 1) * QC],
                                start=True, stop=True,
                                tile_position=(r0, 0))
                        # exp into half of a kt-pair-wide tile; the expB
                        # multiply runs once per pair as a single DVE op
                        if t % 2 == 0:
                            pa_big = pap.tile([128, 4 * QC], BF16, tag="pa",
                                              name="pa")
                            pair_pa[t] = pa_big
                        else:
                            pa_big = pair_pa.pop(t - 1)
                        half = t % 2
                        nc.scalar.activation(
                            pa_big[:, half * 2 * QC:(half + 1) * 2 * QC],
                            pss2, AF.Exp)
                        if t % 2 == 1:
                            kt0 = kt - 1
                            ph_big = php.tile([128, 4 * QC], BF16, tag="ph",
                                              name="ph")
                            pbp = expB[qc][:, kt0 * QC:(kt0 + 2) * QC]
                            nc.vector.tensor_tensor(
                                ph_big.rearrange("p (k i q) -> p k i q",
                                                 k=2, i=2),
                                pa_big.rearrange("p (k i q) -> p k i q",
                                                 k=2, i=2),
                                pbp.rearrange("p (k q) -> p k q",
                                              k=2)[:, :, None, :]
                                   .broadcast_to([128, 2, 2, QC]),
                                OP.mult)
                            ph_ring[t - 1] = ph_big[:, 0:2 * QC]
                            ph_ring[t] = ph_big[:, 2 * QC:4 * QC]

                    def emit_outproj(qc):
                        for st_i in range(QC // 128):
                            s_t = qc * (QC // 128) + st_i
                            ot = otp.tile([128, D], F32, tag="osb",
                                          name="osb")
                            for jc in range(2):
                                psc = projB.tile([128, QC], F32, tag="proj",
                                                 name="proj")
                                for ct in range(NMT):
                                    nc.tensor.matmul(
                                        psc,
                                        OT[ct][qc][:, st_i * 128:
                                                   (st_i + 1) * 128],
                                        wo_sb[ct][:, jc * QC:(jc + 1) * QC],
                                        start=(ct == 0), stop=(ct == NMT - 1))
                                nc.vector.tensor_copy(
                                    ot[:, jc * QC:(jc + 1) * QC], psc)
                            nc.sync.dma_start(
                                out=out_p.ap()[s_t * 128:(s_t + 1) * 128, :],
                                in_=ot)

                    def emit_pv(t):
                        bi, kt = t // NKT, t % NKT
                        qc, hp = blocks[bi]
                        if kt == 0:
                            pso_blk[bi] = [
                                psov.tile([VW, QC], F32, tag="pso",
                                          name="pso") for _ in range(2)]
                        pso = pso_blk[bi]
                        ph2 = ph_ring.pop(t)
                        for i in range(2):
                            h = 2 * hp + i
                            vsl = VH[:, (kt * HPC + h) * VW:
                                     (kt * HPC + h) * VW + VW]
                            nc.tensor.matmul(pso[i], vsl,
                                             ph2[:, i * QC:(i + 1) * QC],
                                             start=(kt == 0),
                                             stop=(kt == NKT - 1))
                        if kt == NKT - 1:
                            rden2 = msp.tile([1, 2 * QC], BF16, tag="rden",
                                             name="rden")
                            with nc.allow_low_precision(
                                    reason="bf16 recip of softmax denom is "
                                           "well within the 2e-2 budget"):
                                for i in range(2):
                                    nc.vector.reciprocal(
                                        rden2[:, i * QC:(i + 1) * QC],
                                        pso[i][HD:HD + 1, :])
                            rbc2 = msp.tile([HD, 2 * QC], BF16, tag="rbc",
                                            name="rbc")
                            nc.gpsimd.partition_broadcast(rbc2, rden2)
                            for i in range(2):
                                nc.vector.tensor_tensor(
                                    OT[hp][qc][i * HD:(i + 1) * HD, :],
                                    pso[i][0:HD, :],
                                    rbc2[:, i * QC:(i + 1) * QC], OP.mult)
                            del pso_blk[bi]
                            done_hp[qc] += 1
                            if done_hp[qc] == NMT:
                                emit_outproj(qc)

                    for t in range(nslots):
                        if t >= LAG:
                            emit_pv(t - LAG)
                        for f in fillers[t]:
                            f()
                        emit_sc(t)
                    for t in range(nslots - LAG, nslots):
                        emit_pv(t)

        if loop_reps:
            with tc.For_i(0, loop_reps, 1):
                body()
        else:
            body()

    nc.finalize()
    return nc


def shard_inputs(inputs):
    """Build per-core in_maps from the full input dict."""
    x = np.asarray(inputs["x"], np.float32)
    dx = np.asarray(inputs["delta_x"], np.float32)
    prior = np.asarray(inputs["prior_mask"], np.float32)
    scl = np.float32(1.0 / np.sqrt(HD))
    wq = np.asarray(inputs["wq"], np.float32) * scl
    bq = np.asarray(inputs["bq"], np.float32) * scl
    wk = np.asarray(inputs["wk"], np.float32)
    bk = np.asarray(inputs["bk"], np.float32)
    wv = np.asarray(inputs["wv"], np.float32)
    bv = np.asarray(inputs["bv"], np.float32)
    wo = np.asarray(inputs["wo"], np.float32)

    bf = ml_dtypes.bfloat16
    priorT = np.ascontiguousarray(prior.T)
    ident = np.eye(128, dtype=np.float32)
    in_maps = []
    for c in range(N_CORES):
        b, g = c // 2, c % 2
        rs = slice(g * GD, (g + 1) * GD)
        in_maps.append({
            "xT": np.ascontiguousarray(x[b].T).astype(bf),
            "dxT": np.ascontiguousarray(dx[b].T).astype(bf),
            "wqT": np.ascontiguousarray(wq[rs].T).astype(bf),
            "wkT": np.ascontiguousarray(wk[rs].T).astype(bf),
            "wvT": np.ascontiguousarray(wv[rs].T).astype(bf),
            "woT": np.ascontiguousarray(wo[:, rs].T).astype(bf),
            "bq": bq[rs].reshape(1, GD).astype(bf),
            "bk": bk[rs].reshape(1, GD).astype(bf),
            "bv": bv[rs].reshape(1, GD).astype(bf),
            "priorT": priorT,
            "ident": ident,
        })
    return in_maps


def assemble_output(inputs, results):
    bo = np.asarray(inputs["bo"], np.float32)
    out = np.empty((B, S, D), np.float32)
    for b in range(B):
        out[b] = results[2 * b]["out_p"] + results[2 * b + 1]["out_p"] + bo
    return out


def kernel(**inputs):
    if "nc" not in _CACHE:
        _CACHE["nc"] = build_nc()
    nc = _CACHE["nc"]
    in_maps = shard_inputs(inputs)
    res = bass_utils.run_bass_kernel_spmd(
        nc, in_maps, core_ids=list(range(N_CORES)), trace=False)
    return assemble_output(inputs, res.results)


# revision 25
# speedup vs baseline: 1.0469x; 1.0469x over previous
"""Trainium2 Bass kernel for ClippingAttentionEngine.

Full (unsharded) inputs in, full output out. Internally shards across 8
NeuronCores: batch (4-way) x head-group (2-way).  Each core computes
attention for one batch and 8 of the 16 heads, plus the row-parallel
partial of the output projection; the host sums the two head-group
partials per batch and adds bo.

Math notes (validated against the reference on the fixed inputs):
 - softmax_k(A + lam*prior) is shift-invariant per query, so the
   threshold subtraction cancels; the clip mask only removes entries
   whose softmax weight is < e^-20 relative to the row max, which is
   below fp32 resolution of the result.  The kernel therefore computes
   plain softmax(QK^T/sqrt(hd) + lam*prior).
 - exp is split as exp(A)*exp(lam*prior): exp(lam*prior) is shared by
   all 8 heads on a core and scaled by the runtime per-batch lam via
   the ACT per-partition scale operand.
 - scores are computed transposed (S^T[k,q]) so P^T feeds the O=P@V
   matmul directly; the softmax denominator rides as an extra ones
   column appended to V (O^T row 64).

Schedule notes (the kernel is ACT(exp)-throughput bound; everything
else is arranged to hide under the exp stream):
 - Phase B runs as one global slot stream over (block, kt) where a
   block is a (qc, hp) pair; the P@V accumulation for slot t-LAG is
   emitted right before the scores matmuls of slot t, so PE never
   waits on the scores->exp->mult chain of the current slot.
 - Blocks are ordered (q0,h0)(q1,h0)(q0,h1)(q1,h1)... so only two
   exp(lam*prior) chunks are resident and the Q/K projections for
   heads 1..3 plus the V projection stream as per-slot fillers inside
   the early blocks instead of serializing ahead of the attention.
 - The lambda chain and the head-0 Q/K projection are paced directly
   behind the x/dx DMA so exp work starts ~25us into the kernel.
 - Denominator reciprocals are broadcast across partitions on the
   otherwise idle GPSIMD engine; the output projection is emitted at
   each qc's drain so it overlaps the next blocks.
"""

import sys

sys.path.insert(0, "/opt/trn_rl_repo")

from contextlib import ExitStack

import numpy as np
import ml_dtypes

import concourse.bacc as bacc
import concourse.tile as tile
from concourse import mybir
from concourse import bass_utils

F32 = mybir.dt.float32
BF16 = mybir.dt.bfloat16
AF = mybir.ActivationFunctionType
OP = mybir.AluOpType
AX = mybir.AxisListType

B, S, D = 4, 2048, 1024
H, HD = 16, 64
N_CORES = 8
HPC = 8          # heads per core
GD = HPC * HD    # head-group width (512)
QC = 512         # q-chunk width
NQC = S // QC    # 4
NKT = S // 128   # 16 k-tiles
NDT = D // 128   # 8 d-tiles
NST = S // 128   # 16 s-tiles
NMT = GD // 128  # 4 m-tiles (head pairs)
VW = HD + 1      # V block width incl. denominator ones column
LAG = 4          # slots the P@V stream trails the scores stream by
LAMBDA_MAX = 10.0
ALPHA = 5.0
EPS = 1e-8

_CACHE = {}


def build_nc(loop_reps=None):
    nc = bacc.Bacc("TRN2", target_bir_lowering=False, debug=False,
                   num_devices=N_CORES)

    xT = nc.dram_tensor("xT", [D, S], BF16, kind="ExternalInput")
    dxT = nc.dram_tensor("dxT", [D, S], BF16, kind="ExternalInput")
    wqT = nc.dram_tensor("wqT", [D, GD], BF16, kind="ExternalInput")
    wkT = nc.dram_tensor("wkT", [D, GD], BF16, kind="ExternalInput")
    wvT = nc.dram_tensor("wvT", [D, GD], BF16, kind="ExternalInput")
    woT = nc.dram_tensor("woT", [GD, D], BF16, kind="ExternalInput")
    bq = nc.dram_tensor("bq", [1, GD], BF16, kind="ExternalInput")
    bk = nc.dram_tensor("bk", [1, GD], BF16, kind="ExternalInput")
    bv = nc.dram_tensor("bv", [1, GD], BF16, kind="ExternalInput")
    priorT = nc.dram_tensor("priorT", [S, S], F32, kind="ExternalInput")
    ident = nc.dram_tensor("ident", [128, 128], F32, kind="ExternalInput")
    out_p = nc.dram_tensor("out_p", [S, D], F32, kind="ExternalOutput")

    with tile.TileContext(nc) as tc, ExitStack() as st_outer:
        consts = st_outer.enter_context(tc.tile_pool(name="consts", bufs=1))
        qkv = st_outer.enter_context(tc.tile_pool(name="qkv", bufs=1))

        ones_row = consts.tile([1, QC], BF16, tag="ones_row")
        nc.vector.memset(ones_row, 1.0)
        ones_c32 = consts.tile([128, 1], F32, tag="ones_c32")
        nc.vector.memset(ones_c32, 1.0)
        ones_r32 = consts.tile([1, 128], F32, tag="ones_r32")
        nc.vector.memset(ones_r32, 1.0)
        ident_sb = consts.tile([128, 128], F32, tag="ident")
        nc.sync.dma_start(out=ident_sb, in_=ident.ap())
        bq_sb = consts.tile([1, GD], BF16, tag="bq")
        nc.sync.dma_start(out=bq_sb, in_=bq.ap())
        bk_sb = consts.tile([1, GD], BF16, tag="bk")
        nc.sync.dma_start(out=bk_sb, in_=bk.ap())
        bv_sb = consts.tile([1, GD], BF16, tag="bv")
        nc.sync.dma_start(out=bv_sb, in_=bv.ap())

        wq_sb = [consts.tile([128, GD], BF16, tag=f"wq{d}", name=f"wq{d}") for d in range(NDT)]
        wk_sb = [consts.tile([128, GD], BF16, tag=f"wk{d}", name=f"wk{d}") for d in range(NDT)]
        wv_sb = [consts.tile([128, GD], BF16, tag=f"wv{d}", name=f"wv{d}") for d in range(NDT)]
        for d in range(NDT):
            nc.sync.dma_start(out=wq_sb[d], in_=wqT.ap()[d * 128:(d + 1) * 128, :])
            nc.sync.dma_start(out=wk_sb[d], in_=wkT.ap()[d * 128:(d + 1) * 128, :])
            nc.sync.dma_start(out=wv_sb[d], in_=wvT.ap()[d * 128:(d + 1) * 128, :])
        wo_sb = [consts.tile([128, D], BF16, tag=f"wo{c}", name=f"wo{c}") for c in range(NMT)]
        for c in range(NMT):
            nc.sync.dma_start(out=wo_sb[c], in_=woT.ap()[c * 128:(c + 1) * 128, :])

        QT = [qkv.tile([128, S], BF16, tag=f"QT{m}", name=f"QT{m}") for m in range(NMT)]
        KT = [qkv.tile([128, S], BF16, tag=f"KT{m}", name=f"KT{m}") for m in range(NMT)]
        VH = qkv.tile([128, NKT * VW * HPC], BF16, tag="VH")
        nc.vector.memset(VH, 1.0)
        OT = [[qkv.tile([128, QC], BF16, tag=f"OT{c}_{q}", name=f"OT{c}_{q}")
               for q in range(NQC)] for c in range(NMT)]
        lam_bc = consts.tile([128, 1], F32, tag="lam_bc")

        def body():
            # ---------------- Phase A: x DMA, head-0 Q/K, lambda ----------
            with tc.tile_pool(name="xp", bufs=1) as xpool:
                x_sb = [xpool.tile([128, S], BF16, tag=f"x{d}", name=f"x{d}")
                        for d in range(NDT)]
                for d in range(NDT):
                    nc.sync.dma_start(out=x_sb[d],
                                      in_=xT.ap()[d * 128:(d + 1) * 128, :])

                # mt=0 Q/K paced behind the x DMA (d-major so the first
                # matmuls only need x0)
                with tc.tile_pool(name="projA", bufs=8, space="PSUM") as projA:
                    psA = [projA.tile([128, QC], F32, tag="projA",
                                      name=f"psA{g}") for g in range(8)]
                    for d in range(NDT):
                        for g in range(8):
                            dst_i, sc = g // 4, g % 4
                            w_sb = wq_sb if dst_i == 0 else wk_sb
                            nc.tensor.matmul(
                                psA[g], w_sb[d][:, 0:128],
                                x_sb[d][:, sc * QC:(sc + 1) * QC],
                                start=(d == 0), stop=False)
                    for g in range(8):
                        dst_i, sc = g // 4, g % 4
                        dst = QT if dst_i == 0 else KT
                        b_sb = bq_sb if dst_i == 0 else bk_sb
                        nc.tensor.matmul(psA[g], b_sb[:, 0:128], ones_row,
                                         start=False, stop=True)
                        nc.vector.tensor_copy(dst[0][:, sc * QC:(sc + 1) * QC],
                                              psA[g])

                # lambda chain: row norms of x and dx via Gram diagonals
                with tc.tile_pool(name="dxp", bufs=1) as dxpool, \
                     tc.tile_pool(name="gr2", bufs=2, space="PSUM") as gr2, \
                     tc.tile_pool(name="tyA", bufs=1, space="PSUM") as tyA, \
                     tc.tile_pool(name="miscA", bufs=2) as miscA:

                    dx_sb = [dxpool.tile([128, S], BF16, tag=f"dxt{d}",
                                         name=f"dxt{d}") for d in range(NDT)]
                    for d in range(NDT):
                        nc.sync.dma_start(
                            out=dx_sb[d],
                            in_=dxT.ap()[d * 128:(d + 1) * 128, :])

                    nx2 = miscA.tile([128, NST], F32, tag="nx2")
                    ndx2 = miscA.tile([128, NST], F32, tag="ndx2")
                    for arr_sb, acc in ((x_sb, nx2), (dx_sb, ndx2)):
                        for s_t in range(NST):
                            psg = gr2.tile([128, 128], F32, tag="gram")
                            for d in range(NDT):
                                sl = arr_sb[d][:, s_t * 128:(s_t + 1) * 128]
                                nc.tensor.matmul(psg, sl, sl, start=(d == 0),
                                                 stop=(d == NDT - 1))
                            diag = miscA.tile([128, 128], F32, tag="diag")
                            nc.vector.tensor_tensor(diag, psg, ident_sb,
                                                    OP.mult)
                            nc.vector.tensor_reduce(acc[:, s_t:s_t + 1], diag,
                                                    axis=AX.X, op=OP.add)

                    nx = miscA.tile([128, NST], F32, tag="nx")
                    ndx = miscA.tile([128, NST], F32, tag="ndx")
                    nc.scalar.activation(nx, nx2, AF.Sqrt)
                    nc.scalar.activation(ndx, ndx2, AF.Sqrt)
                    eps_sb = miscA.tile([128, 1], F32, tag="eps")
                    nc.vector.memset(eps_sb, EPS)
                    nxe = miscA.tile([128, NST], F32, tag="nxe")
                    nc.scalar.activation(nxe, nx, AF.Identity, bias=eps_sb)
                    rx = miscA.tile([128, NST], F32, tag="rx")
                    nc.vector.reciprocal(rx, nxe)
                    u = miscA.tile([128, NST], F32, tag="u")
                    nc.vector.tensor_tensor(u, ndx, rx, OP.mult)
                    usum = miscA.tile([128, 1], F32, tag="usum")
                    nc.vector.tensor_reduce(usum, u, axis=AX.X, op=OP.add)
                    ps_u = tyA.tile([1, 1], F32, tag="psu")
                    nc.tensor.matmul(ps_u, usum, ones_c32, start=True,
                                     stop=True)
                    lam1 = miscA.tile([1, 1], F32, tag="lam1")
                    nc.scalar.activation(lam1, ps_u, AF.Exp, scale=-ALPHA / S)
                    ps_l = tyA.tile([128, 1], F32, tag="psl")
                    nc.tensor.matmul(ps_l, ones_r32, lam1, start=True,
                                     stop=True)
                    nc.scalar.mul(lam_bc, ps_l, LAMBDA_MAX)

                # -------------- Phase B: global slot stream --------------
                # blocks: (qc, hp) in order (q0,h0)(q1,h0)(q0,h1)(q1,h1)...
                # then the q2/q3 section.
                blocks = []
                for qp in range(2):
                    for hp in range(NMT):
                        for j in range(2):
                            blocks.append((2 * qp + j, hp))
                nslots = len(blocks) * NKT

                # fillers[slot] = list of thunks emitting PE work
                fillers = [[] for _ in range(nslots)]

                def emit_v_group(pool, s_t):
                    ps = pool.tile([128, QC], F32, tag="proj", name="proj")
                    for d in range(NDT):
                        nc.tensor.matmul(
                            ps, x_sb[d][:, s_t * 128:(s_t + 1) * 128],
                            wv_sb[d], start=(d == 0), stop=False)
                    nc.tensor.matmul(ps, ones_row[:, 0:128], bv_sb,
                                     start=False, stop=True)
                    base = s_t * VW * HPC
                    dst3 = VH[:, base:base + VW * HPC].rearrange(
                        "p (h c) -> p h c", c=VW)[:, :, 0:HD]
                    src3 = ps.rearrange("p (h c) -> p h c", c=HD)
                    nc.vector.tensor_copy(dst3, src3)

                with tc.tile_pool(name="ps2", bufs=2, space="PSUM") as ps2, \
                     tc.tile_pool(name="psov", bufs=3, space="PSUM") as psov, \
                     tc.tile_pool(name="projB", bufs=1, space="PSUM") as projB, \
                     tc.tile_pool(name="ebp", bufs=2) as ebp, \
                     tc.tile_pool(name="prp", bufs=3) as prp, \
                     tc.tile_pool(name="pap", bufs=2) as pap, \
                     tc.tile_pool(name="php", bufs=3) as php, \
                     tc.tile_pool(name="msp", bufs=2) as msp, \
                     tc.tile_pool(name="otp", bufs=1) as otp:

                    # V projection: one s-tile per slot in block 0
                    for kt in range(NKT):
                        fillers[kt].append(
                            lambda s_t=kt: emit_v_group(projB, s_t))
                    # Q/K mt=1..3: 8 groups each, spread 2/4/4 slots apart
                    for mt, (b0, step) in ((1, (16, 2)), (2, (32, 4)),
                                           (3, (64, 4))):
                        for g in range(8):
                            dst_i, sc = g // 4, g % 4

                            def f(mt=mt, dst_i=dst_i, sc=sc):
                                dst = QT if dst_i == 0 else KT
                                w_sb = wq_sb if dst_i == 0 else wk_sb
                                b_sb = bq_sb if dst_i == 0 else bk_sb
                                ps = projB.tile([128, QC], F32, tag="proj",
                                                name="proj")
                                for d in range(NDT):
                                    nc.tensor.matmul(
                                        ps,
                                        w_sb[d][:, mt * 128:(mt + 1) * 128],
                                        x_sb[d][:, sc * QC:(sc + 1) * QC],
                                        start=(d == 0), stop=False)
                                nc.tensor.matmul(
                                    ps, b_sb[:, mt * 128:(mt + 1) * 128],
                                    ones_row, start=False, stop=True)
                                nc.vector.tensor_copy(
                                    dst[mt][:, sc * QC:(sc + 1) * QC], ps)
                            fillers[b0 + step * g].append(f)

                    expB = {}          # qc -> expB tile
                    ph_ring = {}       # slot -> ph2 tile
                    pso_blk = {}       # block index -> [pso0, pso1]
                    done_hp = {qc: 0 for qc in range(NQC)}

                    def emit_expB(qc):
                        eb = ebp.tile([128, NKT * QC], BF16, tag="expB",
                                      name="expB")
                        for kt in range(NKT):
                            pr = prp.tile([128, QC], F32, tag="prior",
                                          name="prior")
                            nc.sync.dma_start(
                                out=pr,
                                in_=priorT.ap()[kt * 128:(kt + 1) * 128,
                                                qc * QC:(qc + 1) * QC])
                            nc.scalar.activation(
                                eb[:, kt * QC:(kt + 1) * QC], pr, AF.Exp,
                                scale=lam_bc)
                        expB[qc] = eb

                    pair_pa = {}

                    def emit_sc(t):
                        bi, kt = t // NKT, t % NKT
                        qc, hp = blocks[bi]
                        if kt == 0 and qc not in expB:
                            emit_expB(qc)
                        pss2 = ps2.tile([128, 2 * QC], F32, tag="pss2",
                                        name="pss2")
                        for i in range(2):
                            r0 = i * HD
                            nc.tensor.matmul(
                                pss2[:, i * QC:(i + 1) * QC],
                                KT[hp][r0:r0 + HD, kt * 128:(kt + 1) * 128],
                                QT[hp][r0:r0 + HD, qc * QC:(qc + 1) * QC],
                                start=True, stop=True,
                                tile_position=(r0, 0))
                        # exp into half of a kt-pair-wide tile; the expB
                        # multiply runs once per pair as a single DVE op
                        if t % 2 == 0:
                            pa_big = pap.tile([128, 4 * QC], BF16, tag="pa",
                                              name="pa")
                            pair_pa[t] = pa_big
                        else:
                            pa_big = pair_pa.pop(t - 1)
                        half = t % 2
                        nc.scalar.activation(
                            pa_big[:, half * 2 * QC:(half + 1) * 2 * QC],
                            pss2, AF.Exp)
                        if t % 2 == 1:
                            kt0 = kt - 1
                            ph_big = php.tile([128, 4 * QC], BF16, tag="ph",
                                              name="ph")
                            # plain 2D APs per (kt, head) chunk keep DVE in
                            # its fast mode; the broadcast 4D form ran ~2x
                            # slower per element
                            for k in range(2):
                                pbq = expB[qc][:, (kt0 + k) * QC:
                                               (kt0 + k + 1) * QC]
                                for i in range(2):
                                    off = (2 * k + i) * QC
                                    nc.vector.tensor_tensor(
                                        ph_big[:, off:off + QC],
                                        pa_big[:, off:off + QC],
                                        pbq, OP.mult)
                            ph_ring[t - 1] = ph_big[:, 0:2 * QC]
                            ph_ring[t] = ph_big[:, 2 * QC:4 * QC]

                    def emit_outproj(qc):
                        for st_i in range(QC // 128):
                            s_t = qc * (QC // 128) + st_i
                            ot = otp.tile([128, D], F32, tag="osb",
                                          name="osb")
                            for jc in range(2):
                                psc = projB.tile([128, QC], F32, tag="proj",
                                                 name="proj")
                                for ct in range(NMT):
                                    nc.tensor.matmul(
                                        psc,
                                        OT[ct][qc][:, st_i * 128:
                                                   (st_i + 1) * 128],
                                        wo_sb[ct][:, jc * QC:(jc + 1) * QC],
                                        start=(ct == 0), stop=(ct == NMT - 1))
                                nc.vector.tensor_copy(
                                    ot[:, jc * QC:(jc + 1) * QC], psc)
                            nc.sync.dma_start(
                                out=out_p.ap()[s_t * 128:(s_t + 1) * 128, :],
                                in_=ot)

                    def emit_pv(t):
                        bi, kt = t // NKT, t % NKT
                        qc, hp = blocks[bi]
                        if kt == 0:
                            pso_blk[bi] = [
                                psov.tile([VW, QC], F32, tag="pso",
                                          name="pso") for _ in range(2)]
                        pso = pso_blk[bi]
                        ph2 = ph_ring.pop(t)
                        for i in range(2):
                            h = 2 * hp + i
                            vsl = VH[:, (kt * HPC + h) * VW:
                                     (kt * HPC + h) * VW + VW]
                            nc.tensor.matmul(pso[i], vsl,
                                             ph2[:, i * QC:(i + 1) * QC],
                                             start=(kt == 0),
                                             stop=(kt == NKT - 1))
                        if kt == NKT - 1:
                            rden2 = msp.tile([1, 2 * QC], BF16, tag="rden",
                                             name="rden")
                            with nc.allow_low_precision(
                                    reason="bf16 recip of softmax denom is "
                                           "well within the 2e-2 budget"):
                                for i in range(2):
                                    nc.vector.reciprocal(
                                        rden2[:, i * QC:(i + 1) * QC],
                                        pso[i][HD:HD + 1, :])
                            rbc2 = msp.tile([HD, 2 * QC], BF16, tag="rbc",
                                            name="rbc")
                            nc.gpsimd.partition_broadcast(rbc2, rden2)
                            for i in range(2):
                                nc.vector.tensor_tensor(
                                    OT[hp][qc][i * HD:(i + 1) * HD, :],
                                    pso[i][0:HD, :],
                                    rbc2[:, i * QC:(i + 1) * QC], OP.mult)
                            del pso_blk[bi]
                            done_hp[qc] += 1
                            if done_hp[qc] == NMT:
                                emit_outproj(qc)

                    for t in range(nslots):
                        if t >= LAG:
                            emit_pv(t - LAG)
                        for f in fillers[t]:
                            f()
                        emit_sc(t)
                    for t in range(nslots - LAG, nslots):
                        emit_pv(t)

        if loop_reps:
            with tc.For_i(0, loop_reps, 1):
                body()
        else:
            body()

    nc.finalize()
    return nc


def shard_inputs(inputs):
    """Build per-core in_maps from the full input dict."""
    x = np.asarray(inputs["x"], np.float32)
    dx = np.asarray(inputs["delta_x"], np.float32)
    prior = np.asarray(inputs["prior_mask"], np.float32)
    scl = np.float32(1.0 / np.sqrt(HD))
    wq = np.asarray(inputs["wq"], np.float32) * scl
    bq = np.asarray(inputs["bq"], np.float32) * scl
    wk = np.asarray(inputs["wk"], np.float32)
    bk = np.asarray(inputs["bk"], np.float32)
    wv = np.asarray(inputs["wv"], np.float32)
    bv = np.asarray(inputs["bv"], np.float32)
    wo = np.asarray(inputs["wo"], np.float32)

    bf = ml_dtypes.bfloat16
    priorT = np.ascontiguousarray(prior.T)
    ident = np.eye(128, dtype=np.float32)
    in_maps = []
    for c in range(N_CORES):
        b, g = c // 2, c % 2
        rs = slice(g * GD, (g + 1) * GD)
        in_maps.append({
            "xT": np.ascontiguousarray(x[b].T).astype(bf),
            "dxT": np.ascontiguousarray(dx[b].T).astype(bf),
            "wqT": np.ascontiguousarray(wq[rs].T).astype(bf),
            "wkT": np.ascontiguousarray(wk[rs].T).astype(bf),
            "wvT": np.ascontiguousarray(wv[rs].T).astype(bf),
            "woT": np.ascontiguousarray(wo[:, rs].T).astype(bf),
            "bq": bq[rs].reshape(1, GD).astype(bf),
            "bk": bk[rs].reshape(1, GD).astype(bf),
            "bv": bv[rs].reshape(1, GD).astype(bf),
            "priorT": priorT,
            "ident": ident,
        })
    return in_maps


def assemble_output(inputs, results):
    bo = np.asarray(inputs["bo"], np.float32)
    out = np.empty((B, S, D), np.float32)
    for b in range(B):
        out[b] = results[2 * b]["out_p"] + results[2 * b + 1]["out_p"] + bo
    return out


def kernel(**inputs):
    if "nc" not in _CACHE:
        _CACHE["nc"] = build_nc()
    nc = _CACHE["nc"]
    in_maps = shard_inputs(inputs)
    res = bass_utils.run_bass_kernel_spmd(
        nc, in_maps, core_ids=list(range(N_CORES)), trace=False)
    return assemble_output(inputs, res.results)
